# revision 8
# baseline (speedup 1.0000x reference)
"""ConvexSampler Trainium2 kernel.

convex[k] = s[k] * z[idx_i[k]] + (1 - s[k]) * z[idx_j[k]]
out = (concat([z, convex], 0), concat([label_ids, full(UNSEEN)], 0))

Strategy (8 NeuronCores, data-parallel over the 32768 convex samples):
  - core c computes convex rows [c*4096, (c+1)*4096)
  - z (8192x768 f32) is replicated to every core's HBM
  - per core: 32 tiles of 128 rows; each tile gathers its zi / zj rows
    from z with indirect (SWDGE) DMA — hardware honors exactly ONE index
    per partition per instruction, so the offset AP is [128, 1]
  - gathers alternate across SWDGE queues (two Q7 descriptor-generation
    contexts) to parallelize descriptor emission
  - axpy: ACT does (1-s)*zj (per-partition scale), DVE does the fused
    s*zi + that; stores go out on HWDGE

Row->partition layout is "transposed": tile t, partition p holds convex
row p*32 + t, which makes the index / s loads single contiguous DMAs and
keeps every gather/store descriptor at 3 KB.
"""

import os
import numpy as np

import concourse.bacc as bacc
import concourse.bass as bass
import concourse.mybir as mybir
import concourse.tile as tile
from concourse.bass_utils import run_bass_kernel_spmd

N_CORES = 8
BATCH = 8192
FEAT = 768
NUM_CONVEX = 32768
SHARD = NUM_CONVEX // N_CORES  # 4096
P = 128
COLS = SHARD // P  # 32 tiles per core
UNSEEN_LABEL_ID = 150

WORK_BUFS = int(os.environ.get("CONVEX_BUFS", "8"))
N_QUEUES = int(os.environ.get("CONVEX_QUEUES", "2"))

_f32 = mybir.dt.float32
_i32 = mybir.dt.int32


def _set_queue(inst, q):
    if q:
        inst.ins.queue = f"qPoolDynamic{q}"


def build_nc(work_bufs=WORK_BUFS, n_queues=N_QUEUES):
    nc = bacc.Bacc(
        "TRN2", target_bir_lowering=False, debug=False, num_swdge_queues=n_queues
    )

    z = nc.dram_tensor("z", [BATCH, FEAT], _f32, kind="ExternalInput").ap()
    ii = nc.dram_tensor("idx_i", [SHARD], _i32, kind="ExternalInput").ap()
    jj = nc.dram_tensor("idx_j", [SHARD], _i32, kind="ExternalInput").ap()
    ss = nc.dram_tensor("s", [SHARD], _f32, kind="ExternalInput").ap()
    out = nc.dram_tensor("convex", [SHARD, FEAT], _f32, kind="ExternalOutput").ap()

    out3d = out.rearrange("(p n) d -> p n d", p=P)

    with tile.TileContext(nc) as tc:
        with (
            tc.tile_pool(name="idx", bufs=1) as idxp,
            tc.tile_pool(name="work", bufs=work_bufs) as wp,
        ):
            ii_sb = idxp.tile([P, COLS], _i32)
            jj_sb = idxp.tile([P, COLS], _i32)
            ss_sb = idxp.tile([P, COLS], _f32)
            oms_sb = idxp.tile([P, COLS], _f32)  # 1 - s
            nc.sync.dma_start(out=ii_sb[:, :], in_=ii.rearrange("(p n) -> p n", p=P))
            nc.sync.dma_start(out=jj_sb[:, :], in_=jj.rearrange("(p n) -> p n", p=P))
            nc.sync.dma_start(out=ss_sb[:, :], in_=ss.rearrange("(p n) -> p n", p=P))
            nc.vector.tensor_scalar(
                out=oms_sb[:, :], in0=ss_sb[:, :], scalar1=-1.0, scalar2=1.0,
                op0=mybir.AluOpType.mult, op1=mybir.AluOpType.add,
            )

            for t in range(COLS):
                zi = wp.tile([P, FEAT], _f32, tag="zi")
                zj = wp.tile([P, FEAT], _f32, tag="zj")
                ot = wp.tile([P, FEAT], _f32, tag="ot")
                csl = slice(t, t + 1)
                _set_queue(
                    nc.gpsimd.indirect_dma_start(
                        out=zi[:, :], out_offset=None, in_=z,
                        in_offset=bass.IndirectOffsetOnAxis(ap=ii_sb[:, csl], axis=0),
                    ),
                    (2 * t) % n_queues,
                )
                _set_queue(
                    nc.gpsimd.indirect_dma_start(
                        out=zj[:, :], out_offset=None, in_=z,
                        in_offset=bass.IndirectOffsetOnAxis(ap=jj_sb[:, csl], axis=0),
                    ),
                    (2 * t + 1) % n_queues,
                )
                # ot = (1-s)*zj on ACT, then ot = s*zi + ot on DVE
                nc.scalar.mul(ot[:, :], zj[:, :], oms_sb[:, csl])
                nc.vector.scalar_tensor_tensor(
                    out=ot[:, :], in0=zi[:, :], scalar=ss_sb[:, csl],
                    in1=ot[:, :], op0=mybir.AluOpType.mult, op1=mybir.AluOpType.add,
                )
                nc.sync.dma_start(
                    out=out3d[:, csl, :],
                    in_=ot[:, :].rearrange("p (k d) -> p k d", k=1),
                )
    nc.finalize()
    return nc


_NC_CACHE = {}


def _get_nc():
    key = (WORK_BUFS, N_QUEUES)
    if key not in _NC_CACHE:
        _NC_CACHE[key] = build_nc(*key)
    return _NC_CACHE[key]


def make_in_maps(z, idx_i, idx_j, s):
    z = np.ascontiguousarray(np.asarray(z, dtype=np.float32))
    ii = np.ascontiguousarray(np.asarray(idx_i).astype(np.int32))
    jj = np.ascontiguousarray(np.asarray(idx_j).astype(np.int32))
    sv = np.ascontiguousarray(np.asarray(s, dtype=np.float32))
    in_maps = []
    for c in range(N_CORES):
        sl = slice(c * SHARD, (c + 1) * SHARD)
        in_maps.append({"z": z, "idx_i": ii[sl], "idx_j": jj[sl], "s": sv[sl]})
    return in_maps


def kernel(z, label_ids, idx_i, idx_j, s, **bass_run_kwargs):
    label_ids = np.asarray(label_ids)
    nc = _get_nc()
    in_maps = make_in_maps(z, idx_i, idx_j, s)

    res = run_bass_kernel_spmd(
        nc, in_maps, core_ids=list(range(N_CORES)), **bass_run_kwargs
    )
    convex = np.concatenate([r["convex"] for r in res.results], axis=0)

    z32 = np.ascontiguousarray(np.asarray(z, dtype=np.float32))
    z_out = np.concatenate([z32, convex], axis=0)
    labels_out = np.concatenate(
        [label_ids, np.full((NUM_CONVEX,), UNSEEN_LABEL_ID, dtype=label_ids.dtype)]
    )
    kernel.last_results = res
    return z_out, labels_out


kernel.last_results = None
